# revision 32
# baseline (speedup 1.0000x reference)
"""Multi-head attention (B=8, N=1024, D=768, H=12, softmax over full dim-scaled
scores) on 8 Trainium2 NeuronCores, data-parallel over the batch dimension:
core b computes batch element b end-to-end; no collectives.

Per-core pipeline (all matmuls bf16 inputs, fp32 PSUM accumulation):
  1. Host supplies x[b] pre-transposed (feature-major xT [768, 1024]) and the
     weights pre-cast to bf16.
  2. qkT [1536, 1024] = W_qk^T @ x^T   (feature-major q,k)
     v    [1024, 768]  = x @ W_v       (token-major v) + a ones column per head
  3. Per head: scoresT[j, i] = k_h^T q_h  (two heads row-packed on the PE
     array, K=64 each), exp via ScalarE with scale=D^-0.5 folded in,
     out_augT[d+1, i] = [v_h | 1]^T @ expT  -> row 64 is the softmax
     denominator; normalize rows 0..63 by its reciprocal (broadcast via
     GpSimd) into attn_outT [768, 1024].
  4. out = attn_outT^T @ W_out + b_out.
"""

import numpy as np
import ml_dtypes

import concourse.bass as bass
import concourse.bacc as bacc
import concourse.tile as tile
from concourse import mybir
from concourse.bass_utils import run_bass_kernel_spmd

f32 = mybir.dt.float32
bf16 = mybir.dt.bfloat16

B = 8
N = 1024
D = 768
H = 12
DH = 64
SCALE = float(D) ** -0.5
NT = N // 128   # 8 sequence tiles
KT = D // 128   # 6 feature tiles
NPAIR = H // 2  # 6 head pairs


def build_bass():
    nc = bacc.Bacc("TRN2", target_bir_lowering=False, debug=False, num_devices=B)
    xT_d = nc.dram_tensor("xT", [D, N], bf16, kind="ExternalInput")
    wqkv_d = nc.dram_tensor("wqkv", [D, 3 * D], bf16, kind="ExternalInput")
    wo_d = nc.dram_tensor("wo", [D, D], bf16, kind="ExternalInput")
    bo_d = nc.dram_tensor("bo", [D], f32, kind="ExternalInput")
    out_d = nc.dram_tensor("out", [N, D], f32, kind="ExternalOutput")

    with tile.TileContext(nc) as tc:
        with tc.tile_pool(name="persist", bufs=1) as pp:
            # persistent SBUF tensors
            wq_sb = pp.tile([128, KT, 3 * D], bf16)    # W_qkv feature tiles
            xT_sb = pp.tile([128, KT, N], bf16)        # x^T feature tiles
            wo_sb = pp.tile([128, KT, D], bf16)        # W_out feature tiles
            qkT = pp.tile([128, 2 * KT, N], bf16)      # q,k feature-major
            vaug = pp.tile([128, NT, H, DH + 1], bf16)  # v token-major + ones
            aoT = pp.tile([128, KT, N], bf16)          # attention out, feature-major
            bias_f32 = pp.tile([1, D], f32)
            bias_bf = pp.tile([1, D], bf16)
            ones_col = pp.tile([1, 128], bf16)

            # ---- input DMAs. Each dma_start costs ~0.6 us of issue time on
            # its issuing engine, so the startup-critical ones are spread
            # across three engines (sync/gpsimd/vector) to overlap issue.
            for kt in range(KT):
                nc.sync.dma_start(
                    out=xT_sb[:, kt, :], in_=xT_d[kt * 128:(kt + 1) * 128, :]
                )
            for kt in range(KT):  # pair-0 q columns
                nc.gpsimd.dma_start(
                    out=wq_sb[:, kt, 0:128],
                    in_=wqkv_d[kt * 128:(kt + 1) * 128, 0:128],
                )
            for kt in range(KT):  # pair-0 k columns
                nc.scalar.dma_start(
                    out=wq_sb[:, kt, KT * 128:(KT + 1) * 128],
                    in_=wqkv_d[kt * 128:(kt + 1) * 128, KT * 128:(KT + 1) * 128],
                )
            for kt in range(KT):  # v columns (pair-0 fillers)
                nc.scalar.dma_start(
                    out=wq_sb[:, kt, 2 * D:], in_=wqkv_d[kt * 128:(kt + 1) * 128, 2 * D:]
                )
            for kt in range(KT):  # q rest
                nc.sync.dma_start(
                    out=wq_sb[:, kt, 128:D],
                    in_=wqkv_d[kt * 128:(kt + 1) * 128, 128:D],
                )
            for kt in range(KT):  # k rest
                nc.sync.dma_start(
                    out=wq_sb[:, kt, D + 128:2 * D],
                    in_=wqkv_d[kt * 128:(kt + 1) * 128, D + 128:2 * D],
                )
            for kt in range(KT):
                nc.sync.dma_start(
                    out=wo_sb[:, kt, :], in_=wo_d[kt * 128:(kt + 1) * 128, :]
                )
            bo_ap = bo_d[:]
            nc.sync.dma_start(
                out=bias_f32,
                in_=bass.AP(tensor=bo_ap.tensor, offset=bo_ap.offset,
                            ap=[[0, 1]] + list(bo_ap.ap)),
            )
            nc.vector.tensor_copy(out=bias_bf, in_=bias_f32)
            nc.gpsimd.memset(ones_col, 1.0)
            nc.vector.memset(vaug[:, :, :, DH], 1.0)
            # dummy activation: pulls the exp ACT-table load (~2.7 us) into
            # the initial DMA wait instead of the first real exp
            warm = pp.tile([1, 2], f32)
            nc.vector.memset(warm, 0.0)
            nc.scalar.activation(out=warm, in_=warm,
                                 func=mybir.ActivationFunctionType.Exp)

            # ---- stages B+C: qkv projections interleaved with attention.
            # The attention phase is ScalarE-bound (exp): ~16 us per head
            # pair while PE only has ~7 us of scores+PV work. Feeding PE the
            # v / next-pair qk projection matmuls inside the pair loop keeps
            # it busy (otherwise HAM re-throttles it to 1.2 GHz).
            with tc.tile_pool(name="sbC", bufs=4) as sbC, \
                 tc.tile_pool(name="sbAug", bufs=6) as sbAug, \
                 tc.tile_pool(name="sbCs", bufs=2) as sbCs, \
                 tc.tile_pool(name="psQK", bufs=2, space="PSUM") as psQK, \
                 tc.tile_pool(name="psS", bufs=1, space="PSUM") as psS, \
                 tc.tile_pool(name="psV", bufs=2, space="PSUM") as psV:

                def emit_qk_chunk(m, it):
                    # qkT[:, m, it-half] = W_qk[:, m-cols]^T @ x^T
                    ps = psQK.tile([128, 512], f32, tag="proj")
                    for kt in range(KT):
                        nc.tensor.matmul(
                            ps,
                            wq_sb[:, kt, m * 128:(m + 1) * 128],
                            xT_sb[:, kt, it * 512:(it + 1) * 512],
                            start=(kt == 0), stop=(kt == KT - 1),
                        )
                    nc.vector.tensor_copy(
                        out=qkT[:, m, it * 512:(it + 1) * 512], in_=ps
                    )

                def emit_v_chunk(jt, et):
                    # v[jt-tile, 6 heads] = x @ W_v  (+ strided head layout)
                    ps = psQK.tile([128, 384], f32, tag="proj")
                    for kt in range(KT):
                        nc.tensor.matmul(
                            ps,
                            xT_sb[:, kt, jt * 128:(jt + 1) * 128],
                            wq_sb[:, kt, 2 * D + et * 384: 2 * D + (et + 1) * 384],
                            start=(kt == 0), stop=(kt == KT - 1),
                        )
                    nc.vector.tensor_copy(
                        out=vaug[:, jt, 6 * et:6 * (et + 1), 0:DH],
                        in_=ps.rearrange("p (h d) -> p h d", d=DH),
                    )

                def emit_pv_group(q, parity, it, st):
                    # one softmax-numerator matmul group of pair q:
                    # out_augT[d+1, i-half] = [v_h | 1]^T @ expT_h
                    e = st["eA"] if parity == 0 else st["eB"]
                    h = 2 * q + parity
                    ops = psV.tile([DH + 1, 512], f32, tag="pv")
                    for jt_ in range(NT):
                        nc.tensor.matmul(
                            ops,
                            vaug[:, jt_, h, :],
                            e[:, jt_, it * 512:(it + 1) * 512],
                            start=(jt_ == 0), stop=(jt_ == NT - 1),
                        )
                    aug = sbAug.tile([DH + 1, 512], f32, tag="aug")
                    nc.vector.tensor_copy(out=aug, in_=ops)
                    idx = 2 * parity + it
                    nc.vector.tensor_copy(
                        out=st["s4"][32 * idx:32 * idx + 1, :],
                        in_=aug[DH:DH + 1, :],
                    )
                    st["augs"][idx] = aug

                def emit_pv_tail(q, st):
                    # batched reciprocal of the pair's 4 softmax denominators
                    # (rows 0/32/64/96 of s4), then broadcast + normalize.
                    r4 = sbCs.tile([97, 512], f32, tag="r4")
                    nc.vector.reciprocal(out=r4, in_=st["s4"])
                    for parity in (0, 1):
                        for it in range(2):
                            idx = 2 * parity + it
                            rr = sbCs.tile([1, 512], f32, tag="rr")
                            nc.vector.tensor_copy(
                                out=rr, in_=r4[32 * idx:32 * idx + 1, :]
                            )
                            rbc = sbCs.tile([DH, 512], f32, tag="rsbc")
                            nc.gpsimd.partition_broadcast(rbc, rr)
                            nc.vector.tensor_mul(
                                out=aoT[parity * DH:(parity + 1) * DH, q,
                                        it * 512:(it + 1) * 512],
                                in0=st["augs"][idx][0:DH, :],
                                in1=rbc,
                            )

                # head-start: just q,k of pair 0 — the first scores as early
                # as possible; all of v becomes pair-0 filler.
                for it in range(2):
                    emit_qk_chunk(0, it)
                for it in range(2):
                    emit_qk_chunk(KT + 0, it)

                # Software-pipelined pair loop: iteration p runs the scores +
                # exp of pair p on PE/ACT while interleaving (a) the PV matmul
                # groups of pair p-1 and (b) projection chunks for pair p+1 /
                # the rest of v, so the in-order PE queue always has ready
                # work ahead of the next psum-slot-gated scores matmul.
                prev_st = None
                for p in range(NPAIR + 1):
                    cur = p if p < NPAIR else None
                    filler = []
                    if p == 0:
                        filler = [("v", jt, et) for jt in range(NT) for et in range(2)]
                    if cur is not None and p + 1 < NPAIR:
                        filler += [("qk", p + 1, it) for it in range(2)]
                        filler += [("qk", KT + p + 1, it) for it in range(2)]

                    cur_st = None
                    if cur is not None:
                        cur_st = {
                            "eA": sbC.tile([128, NT, N], bf16, tag="expT", name="eA"),
                            "eB": sbC.tile([128, NT, N], bf16, tag="expT", name="eB"),
                            "s4": sbCs.tile([97, 512], f32, tag="s4", name="s4"),
                            "augs": {},
                        }
                    if cur is None:
                        # drain: the last pair's PV groups, then the bulk of
                        # the output projection (ct 0..4 — independent of the
                        # last pair's normalization) fills PE while the
                        # reciprocal/broadcast/mul chain runs on DVE/GpSimd.
                        for parity in (0, 1):
                            for it in range(2):
                                emit_pv_group(p - 1, parity, it, prev_st)
                        emit_pv_tail(p - 1, prev_st)
                        break

                    def emit_filler(k):
                        kind, a1, a2 = filler[k]
                        if kind == "v":
                            emit_v_chunk(a1, a2)
                        else:
                            emit_qk_chunk(a1, a2)

                    fi = 0
                    pv_slots = {1: (0, 0), 3: (0, 1), 5: (1, 0), 7: (1, 1)}
                    for jt in range(NT):
                        # 1) interleaved PV group of the previous pair (keeps
                        #    PE fed while exp(jt-1) finishes)
                        if prev_st is not None and jt in pv_slots:
                            parity, it = pv_slots[jt]
                            emit_pv_group(p - 1, parity, it, prev_st)
                        # 2) filler projection chunks: one ahead of the
                        #    scores (PE cover for exp(jt-1)), rest after
                        n_take = ((jt + 1) * len(filler)) // NT - fi
                        if prev_st is None and n_take > 0:
                            emit_filler(fi)
                            fi += 1
                            n_take -= 1
                        # 3) scores + exp of the current pair
                        sA = psS.tile([128, N], f32, tag="sA")
                        sB = psS.tile([128, N], f32, tag="sB")
                        for it in range(2):
                            # scoresT[j, i] = sum_d k[d, j] q[d, i]; heads 2p
                            # (partitions 0:64) and 2p+1 (64:128) row-packed
                            nc.tensor.matmul(
                                sA[:, it * 512:(it + 1) * 512],
                                qkT[0:DH, KT + p, jt * 128:(jt + 1) * 128],
                                qkT[0:DH, p, it * 512:(it + 1) * 512],
                                start=True, stop=True,
                            )
                            nc.tensor.matmul(
                                sB[:, it * 512:(it + 1) * 512],
                                qkT[DH:128, KT + p, jt * 128:(jt + 1) * 128],
                                qkT[DH:128, p, it * 512:(it + 1) * 512],
                                start=True, stop=True,
                            )
                        nc.scalar.activation(
                            out=cur_st["eA"][:, jt, :], in_=sA,
                            func=mybir.ActivationFunctionType.Exp, scale=SCALE,
                        )
                        nc.scalar.activation(
                            out=cur_st["eB"][:, jt, :], in_=sB,
                            func=mybir.ActivationFunctionType.Exp, scale=SCALE,
                        )
                        # 4) remaining filler chunks for this jt
                        for _ in range(n_take):
                            emit_filler(fi)
                            fi += 1
                    if prev_st is not None:
                        emit_pv_tail(p - 1, prev_st)
                    prev_st = cur_st

            # ---- stage D: output projection in two passes.
            # Pass 1 (ct 0..4 + rank-1 bias matmul) only needs pairs 0..4, so
            # its matmuls keep PE busy while the last pair's normalization
            # chain finishes on DVE/GpSimd; ScalarE (idle here) evacuates the
            # partials. Pass 2 adds the ct=5 product and stores.
            with tc.tile_pool(name="sbD", bufs=1) as sbD, \
                 tc.tile_pool(name="sbDo", bufs=3) as sbDo, \
                 tc.tile_pool(name="psD", bufs=2, space="PSUM") as psD, \
                 tc.tile_pool(name="psD2", bufs=4, space="PSUM") as psD2:
                osb_all = sbD.tile([128, NT, 2, 384], f32)
                for nt in range(NT):
                    for et in range(2):
                        ps = psD.tile([128, 384], f32, tag="ops")
                        nc.tensor.matmul(
                            ps, ones_col,
                            bias_bf[:, et * 384:(et + 1) * 384],
                            start=True, stop=False,
                        )
                        for ct in range(KT - 1):
                            nc.tensor.matmul(
                                ps,
                                aoT[:, ct, nt * 128:(nt + 1) * 128],
                                wo_sb[:, ct, et * 384:(et + 1) * 384],
                                start=False, stop=(ct == KT - 2),
                            )
                        nc.scalar.copy(out=osb_all[:, nt, et, :], in_=ps)
                for nt in range(NT):
                    osb = sbDo.tile([128, D], f32, tag="osb")
                    for et in range(2):
                        ps = psD2.tile([128, 384], f32, tag="ops2")
                        ct = KT - 1
                        nc.tensor.matmul(
                            ps,
                            aoT[:, ct, nt * 128:(nt + 1) * 128],
                            wo_sb[:, ct, et * 384:(et + 1) * 384],
                            start=True, stop=True,
                        )
                        nc.vector.tensor_add(
                            out=osb[:, et * 384:(et + 1) * 384],
                            in0=ps,
                            in1=osb_all[:, nt, et, :],
                        )
                        # store each half as soon as its add lands
                        nc.sync.dma_start(
                            out=out_d[nt * 128:(nt + 1) * 128,
                                      et * 384:(et + 1) * 384],
                            in_=osb[:, et * 384:(et + 1) * 384],
                        )
    nc.compile()
    return nc


_CACHE = {}


def _get_nc():
    if "nc" not in _CACHE:
        _CACHE["nc"] = build_bass()
    return _CACHE["nc"]


def _make_in_maps(x, w_qkv, w_out, b_out):
    bf = ml_dtypes.bfloat16
    x = np.asarray(x, dtype=np.float32)
    wq_bf = np.ascontiguousarray(np.asarray(w_qkv, dtype=np.float32)).astype(bf)
    wo_bf = np.ascontiguousarray(np.asarray(w_out, dtype=np.float32)).astype(bf)
    bo = np.ascontiguousarray(np.asarray(b_out, dtype=np.float32))
    in_maps = []
    for b in range(B):
        xT = np.ascontiguousarray(x[b].T).astype(bf)
        in_maps.append({"xT": xT, "wqkv": wq_bf, "wo": wo_bf, "bo": bo})
    return in_maps


def kernel(x, w_qkv, w_out, b_out):
    nc = _get_nc()
    in_maps = _make_in_maps(x, w_qkv, w_out, b_out)
    res = run_bass_kernel_spmd(nc, in_maps, list(range(B)))
    return np.stack([res.results[b]["out"] for b in range(B)]).astype(np.float32)


# ---------------------------------------------------------------------------
# profiling helper (used by test.py only; safe no-op fallback if the axon
# NTFF hook infrastructure is unavailable)
def _install_profhook():
    import sys
    import types

    if "antenv.axon_hooks" in sys.modules:
        return True
    try:
        import antenv
        from trn_agent_boot.trn_boot import _ntff_profile_via_ctypes

        hook = _ntff_profile_via_ctypes("/opt/axon/libaxon_pjrt.so")
        mod = types.ModuleType("antenv.axon_hooks")
        mod._hook = hook
        mod.get_axon_ntff_profile_hook = lambda: mod._hook

        def _set(h):
            mod._hook = h

        mod.set_axon_ntff_profile_hook = _set
        sys.modules["antenv.axon_hooks"] = mod
        antenv.axon_hooks = mod

        import concourse.bass_utils as bu

        bu.upload_artifacts = lambda tmpdir: f"local:{tmpdir}"
        return True
    except Exception as e:  # pragma: no cover
        print(f"profhook install failed: {e}")
        return False


def run_traced(x, w_qkv, w_out, b_out, tmpdir=None):
    """Run with NTFF profiling; returns (out, exec_time_ns, results_obj)."""
    traced = _install_profhook()
    nc = _get_nc()
    in_maps = _make_in_maps(x, w_qkv, w_out, b_out)
    res = run_bass_kernel_spmd(
        nc, in_maps, list(range(B)), trace=traced, tmpdir=tmpdir
    )
    out = np.stack([res.results[b]["out"] for b in range(B)]).astype(np.float32)
    return out, res.exec_time_ns, res
